# revision 1
# baseline (speedup 1.0000x reference)
"""DirectionalConv3d Trainium2 kernel (v2: bf16 HBM I/O + 4-quadrant PE).

out[b, o, t, r, c] = sum_d W_d[o, :] . x[b, :, (t,r,c)+delta_d]
for the 7-point directional stencil (self, t+-1, r+-1, c+-1), zero padded.

Strategy (per core, 1 batch per core, 8 cores):
  - Host pre-casts x to bf16 and pads each row to pitch 34 (2 zero cols), so
    HBM read is ~4.7MB instead of 9.4MB and c+-1 shifted reads hit zeros at
    row boundaries with no on-chip restaging: DMA lands the image directly
    in SBUF in compute layout.  Output is written bf16 (dense) and upcast
    host-side: write ~4.2MB instead of 8.4MB.
  - T-halved image: partitions 0-63 hold planes 0-16 (+ a zero halo slot in
    front), partitions 64-127 hold planes 15-31 (+ zero halo at back).  Both
    halves use identical local slot indexing (slot = local_out_plane+1+dt).
  - 4 concurrent PE tile-position streams: quadrant (lhs_half*64,
    psum_half*64) computes one output plane per round; rounds r=0..7 cover
    planes (2r, 2r+1, 16+2r, 17+2r).  Each plane = 7 directional matmuls x
    2 psum chunks (out rows 0-15 / 16-31, 512 f32 = 1 bank each).  r+-1
    shifts clip the out-row range instead of padding (skipped rows == +0).
  - PSUM: 4 bank-tiles per round (2 x-halves x 2 row-chunks; top/bottom
    psum partition halves hold the even/odd plane), double buffered = 8.
  - d-loop order: self first (start=True full bank), cm last (stop=True
    full bank) so every psum element follows start..stop discipline.
  - LDWEIGHTS dedup: a post-pass sets ldweights=False on any matmul whose
    tile position already holds the same stationary.
  - Evac: one cast-copy per psum half-tile (f32->bf16) alternating DVE /
    Activation, into a [128, SGRP*2048] stage (top partitions = half0
    planes, bottom = half1), then 2 dense bf16 DMAs per SGRP rounds.
"""

import numpy as np
import ml_dtypes
import os

B = 8
CI = 64
CO = 64
T = 32
R = 32
C = 32
RP = 34                  # padded row pitch (2 zero cols)
PL = R * RP              # 1088 elems per padded plane (input image)
OPL = R * C              # 1024 elems per dense output plane
N = T * OPL              # dense output elems per channel
LEAD = 2                 # zero guard elems at each image-chunk tile front
SGRP = int(os.environ.get("KERNEL_SGRP", "2"))       # rounds per out-DMA
# small first chunks so round 0 compute starts as early as possible
CHUNK_SLOTS = (2, 2, 4, 4, 3, 3)                      # image slot chunking

# direction -> (dt, dr, dc); order fixed: self first (start), cm last (stop)
DIRS = [
    ("self", 0, 0, 0),
    ("rp", 0, -1, 0),
    ("rm", 0, 1, 0),
    ("cp", 0, 0, -1),
    ("tp", -1, 0, 0),
    ("tm", 1, 0, 0),
    ("cm", 0, 0, 1),
]
NW = len(DIRS)

_NC_CACHE = {}


def _chunk_of_slot(slot):
    s0 = 0
    for ci, n in enumerate(CHUNK_SLOTS):
        if s0 <= slot < s0 + n:
            return ci, s0, n
        s0 += n
    raise AssertionError(slot)


def _emit(nc, tc, x, wt, out, mybir, bass):
    bf16 = mybir.dt.bfloat16
    AP = bass.AP

    xpool = tc.alloc_tile_pool(name="xin", bufs=1)
    wpool = tc.alloc_tile_pool(name="wp", bufs=1)
    apool = tc.alloc_tile_pool(name="accp", bufs=2, space="PSUM")
    spool = tc.alloc_tile_pool(name="stg", bufs=3)

    # ---- weights [7,64,64]: lhsT layout host-side, loaded to both halves --
    w_sb = wpool.tile([128, NW * CO], bf16, name="w_sb")
    w_src = wt.transpose([1, 0, 2])  # [i, d, o]
    nc.sync.dma_start(out=w_sb[0:64, :], in_=w_src)
    nc.sync.dma_start(out=w_sb[64:128, :], in_=w_src)

    # ---- image chunks: DMA bf16 padded planes straight into SBUF ----
    # half h slot k holds: h0: plane k-1 (slot0 = zeros), h1: plane 15+k
    # (slot17 = zeros).  Two 64-partition DMAs per chunk (complementary
    # DMA-engine fan-out, see measured note in the baseline kernel).
    xts = []
    s0 = 0
    for ci, nsl in enumerate(CHUNK_SLOTS):
        # +2 tail: the rearrange view of a cm-shifted read of the last row
        # spans (but never reads) up to 2 elems past the last plane
        xt = xpool.tile([128, LEAD + nsl * PL + 2], bf16, name=f"xc{ci}")
        # top half: slots s0..s0+nsl-1 -> planes s0-1..s0+nsl-2
        p_lo = s0 - 1
        if p_lo < 0:
            nc.vector.memset(xt[0:64, 0:LEAD + PL], 0.0)
            top_dst = xt[0:64, LEAD + PL:LEAD + nsl * PL]
            top_src = AP(x.tensor, 0, [[T * PL, CI], [1, (nsl - 1) * PL]])
        else:
            nc.vector.memset(xt[0:64, 0:LEAD], 0.0)
            top_dst = xt[0:64, LEAD:LEAD + nsl * PL]
            top_src = AP(x.tensor, p_lo * PL, [[T * PL, CI], [1, nsl * PL]])
        nc.sync.dma_start(out=top_dst, in_=top_src)
        # bottom half: slots s0.. -> planes 15+s0..
        q_lo = 15 + s0
        q_hi = 15 + s0 + nsl
        if q_hi > T:
            ndma = T - q_lo
            nc.vector.memset(xt[64:128, 0:LEAD], 0.0)
            nc.vector.memset(xt[64:128, LEAD + ndma * PL:], 0.0)
            bot_dst = xt[64:128, LEAD:LEAD + ndma * PL]
            bot_src = AP(x.tensor, q_lo * PL, [[T * PL, CI], [1, ndma * PL]])
        else:
            nc.vector.memset(xt[64:128, 0:LEAD], 0.0)
            bot_dst = xt[64:128, LEAD:LEAD + nsl * PL]
            bot_src = AP(x.tensor, q_lo * PL, [[T * PL, CI], [1, nsl * PL]])
        nc.sync.dma_start(out=bot_dst, in_=bot_src)
        xts.append(xt)
        s0 += nsl

    def w_ap(di, h):
        return w_sb[h * 64:(h + 1) * 64, di * CO:(di + 1) * CO]

    def rhs_ap(h, slot, xrow0, nrows, dc):
        """rhs AP: x rows xrow0..xrow0+nrows, cols dc..dc+32 of a slot."""
        ci, cs0, nsl = _chunk_of_slot(slot)
        xt = xts[ci]
        lo = h * 64
        start = LEAD + (slot - cs0) * PL + xrow0 * RP + dc
        v = xt[lo:lo + 64, start:start + nrows * RP]
        v = v.rearrange("p (r c) -> p r c", c=RP)
        return v[:, :, 0:C]

    # ---- main loop: 8 rounds x 7 dirs x 2 row-chunks x 4 quadrants ----
    for r in range(8):
        # psum tiles: one 2-bank tile per x-half; top/bottom psum partition
        # halves hold the even/odd output plane of this round
        ptiles = {}
        for h in range(2):
            ptiles[h] = apool.tile(
                [128, 1024], mybir.dt.float32, name=f"ps{r}_{h}",
                tag=f"ps{h}")

        # alternate direction order per round: the boundary direction's
        # stationary stays loaded, saving 4 LDWEIGHTS per round boundary
        order = range(NW) if r % 2 == 0 else range(NW - 1, -1, -1)
        for k, di in enumerate(order):
            dname, dt_, dr, dc = DIRS[di]
            first = k == 0
            last = k == NW - 1
            for c2 in range(2):
                row0 = c2 * 16
                # out rows valid iff 0 <= row+dr < R
                orow0 = max(row0, -dr)
                orow1 = min(row0 + 16, R - dr)
                for h in range(2):          # x-half = lhsT partition half
                    for pb in range(2):     # psum partition half
                        po = 2 * r + pb     # local out plane 0..15
                        slot = po + 1 + dt_
                        lo = pb * 64
                        acc = ptiles[h]
                        cb = c2 * 512       # bank base inside 2-bank tile
                        if first or last:   # dr == 0: full bank
                            oap = acc[lo:lo + 64, cb:cb + 512]
                            rhs = rhs_ap(h, slot, row0, 16, dc)
                        else:
                            oap = acc[lo:lo + 64,
                                      cb + (orow0 - row0) * C:
                                      cb + (orow1 - row0) * C]
                            rhs = rhs_ap(h, slot, orow0 + dr,
                                         orow1 - orow0, dc)
                        nc.tensor.matmul(
                            out=oap, lhsT=w_ap(di, h), rhs=rhs,
                            start=first, stop=last,
                            # sim psum-group tracker aliases partition
                            # ranges within a bank; per-element on HW
                            skip_group_check=True)

        # ---- evac round r: 4 plane cast-copies into bf16 stage ----
        if r % SGRP == 0:
            stage = spool.tile([128, SGRP * 2 * OPL], bf16,
                               name=f"st{r}", tag="st")
        soff = (r % SGRP) * 2 * OPL
        for h in range(2):
            for pb in range(2):
                src = ptiles[h][pb * 64:pb * 64 + 64, :]
                dst = stage[h * 64:h * 64 + 64,
                            soff + pb * OPL:soff + pb * OPL + OPL]
                if (h + pb) % 2 == 0:
                    nc.vector.tensor_copy(out=dst, in_=src)
                else:
                    nc.scalar.copy(out=dst, in_=src)
        if r % SGRP == SGRP - 1:
            p0 = (r - (SGRP - 1)) * 2
            dst_top = AP(out.tensor, p0 * OPL, [[N, CO], [1, SGRP * 2 * OPL]])
            dst_bot = AP(out.tensor, (16 + p0) * OPL,
                         [[N, CO], [1, SGRP * 2 * OPL]])
            # out-DMAs ride the scalar/vector DGE rings so they never queue
            # behind (or delay) the in-DMA stream on the sync ring
            nc.scalar.dma_start(out=dst_top, in_=stage[0:64, :])
            nc.scalar.dma_start(out=dst_bot, in_=stage[64:128, :])

    for p in (spool, apool, wpool, xpool):
        p.release()


def _dedup_ldweights(nc, mybir):
    """Drop InstLdweights whose tile position already holds the same
    stationary (the PE keeps per-quadrant weights until overwritten).
    Any semaphore waits on a dropped load are preserved on an InstNoOp in
    its place on the PE queue."""
    SyncInfo = mybir.SyncInfo
    counter = [0]
    last_w = {}
    for f in nc.m.functions:
        for blk in f.blocks:
            out, changed = [], False
            for inst in blk.instructions:
                if type(inst).__name__ != "InstLdweights":
                    out.append(inst)
                    continue
                pos = tuple(inst.tile_position or (0, 0))
                wap = inst.ins[0]
                key = (wap.memref, wap.offset,
                       tuple(tuple(p) for p in wap.ap))
                si = getattr(inst, "sync_info", None)
                ups = list(si.on_update) if si is not None and si.on_update \
                    else []
                if last_w.get(pos) == key and not ups:
                    waits = list(si.on_wait) if si is not None and \
                        si.on_wait else []
                    if waits:
                        nop = mybir.InstNoOp(name=f"ldwnop_{counter[0]}")
                        counter[0] += 1
                        nop.engine = inst.engine
                        nop.sync_info = SyncInfo(on_wait=waits, on_update=[])
                        nc.register_instruction(nop, overwrite=True)
                        out.append(nop)
                    changed = True
                    continue
                last_w[pos] = key
                out.append(inst)
            if changed:
                blk.instructions = out


def _split_multi_waits(nc, mybir):
    """Walrus codegen allows only one sem-wait slot per engine instruction
    ("Too many sync wait commands").  Hoist all but one wait of any
    multi-wait instruction onto InstNoOp's inserted immediately before it
    on the same engine queue — semantically identical for in-order
    engines (the nop blocks the queue until its wait passes)."""
    SyncInfo = mybir.SyncInfo
    counter = [0]
    for blk in nc.m.functions[0].blocks:
        insts = list(blk.instructions)
        out, changed = [], False
        for inst in insts:
            si = getattr(inst, "sync_info", None)
            waits = list(si.on_wait) if si is not None and si.on_wait else []
            if len(waits) > 1:
                for w in waits[:-1]:
                    nop = mybir.InstNoOp(name=f"waitnop_{counter[0]}")
                    counter[0] += 1
                    nop.engine = inst.engine
                    nop.sync_info = SyncInfo(on_wait=[w], on_update=[])
                    nc.register_instruction(nop, overwrite=True)
                    out.append(nop)
                si.on_wait = [waits[-1]]
                changed = True
            out.append(inst)
        if changed:
            blk.instructions = out


def build_nc():
    import concourse.bass as bass
    import concourse.mybir as mybir
    import concourse.tile as tile

    key = (SGRP,)
    if key in _NC_CACHE:
        return _NC_CACHE[key]
    nc = bass.Bass("TRN2", target_bir_lowering=False, debug=False)
    x = nc.dram_tensor("x", [CI, T * PL], mybir.dt.bfloat16,
                       kind="ExternalInput").ap()
    wt = nc.dram_tensor("wt", [NW, CI, CO], mybir.dt.bfloat16,
                        kind="ExternalInput").ap()
    out = nc.dram_tensor("out", [CO, N], mybir.dt.bfloat16,
                         kind="ExternalOutput").ap()
    with tile.TileContext(nc) as tc:
        _emit(nc, tc, x, wt, out, mybir, bass)
    _dedup_ldweights(nc, mybir)
    _split_multi_waits(nc, mybir)
    _NC_CACHE[key] = nc
    return nc


def host_x(xb):
    """Pad [CI, T, R, C] f32 -> [CI, T*R*RP] bf16 with zero pad cols."""
    xp = np.zeros((CI, T, R, RP), dtype=ml_dtypes.bfloat16)
    xp[:, :, :, 0:C] = xb
    return np.ascontiguousarray(xp.reshape(CI, T * PL))


def host_weights(inputs):
    """Stack + transpose the weights into lhsT layout [7, i, o] bf16,
    ordered as DIRS."""
    names = {"self": "w_self", "tp": "w_tp", "tm": "w_tm", "rp": "w_rp",
             "rm": "w_rm", "cp": "w_cp", "cm": "w_cm"}
    ws = [np.asarray(inputs[names[d[0]]], dtype=np.float32) for d in DIRS]
    wt = np.stack([np.ascontiguousarray(w.T) for w in ws])
    return wt.astype(ml_dtypes.bfloat16)


def kernel(**inputs):
    from concourse.bass_utils import run_bass_kernel_spmd

    nc = build_nc()
    x = np.asarray(inputs["x"], dtype=np.float32)
    wt = host_weights(inputs)
    in_maps = [
        {"x": host_x(x[b]), "wt": wt}
        for b in range(B)
    ]
    res = run_bass_kernel_spmd(nc, in_maps, list(range(B))).results
    out = np.stack([np.asarray(res[b]["out"], dtype=np.float32)
                    .reshape(CO, T, R, C) for b in range(B)])
    return out

